# revision 9
# baseline (speedup 1.0000x reference)
"""DifferentialAttention (B=2, S=2048, D=2048, H=16, KVH=8) on 8 TRN2 NeuronCores.

Sharding: 8 cores = 2 (batch) x 4 (tensor-parallel head groups), as baseline.

v1 rewrite (cost-model-driven):
  - bf16 inputs/weights (host-cast): halves DMA, same matmul rate (1.0 cyc/row)
  - all weights resident in SBUF, loaded once
  - RoPE rotate-half via ONE stream_shuffle (head dims host-permuted so the
    rotate partner sits +/-16 within the same 32-partition quadrant)
  - k head duplication via direct DVE half-adds (no sbuf->sbuf DMAs)
  - scores S1,S2 -> one 2-bank psum tile [128,1024]; ONE merged exp per ki
  - causal mask as -50 bias added by a PE matmul (maskT x identC) into psum,
    so exp gives ~0; no DVE mask multiplies
  - softmax denominators R1,R2 via skinny N=1 matmuls (ET chunks as lhsT,
    ones column rhs) accumulated in a small psum tile -> q-partition layout;
    per-q scalars computed on [128,4] tiles; partition broadcast via PE
    transpose + bf16 outer-product matmuls (cost ~53ns each)
  - eps term: pre = var + 128*eps*R1^2 in q-layout (skinny var matmuls too)
  - rsqrt deferred: ALL pairs' pre collected in [128,64]; ONE Ln + ONE Exp
    at end of phase B (2 act-table loads total instead of 33)
  - o_proj: bf16 otf x resident bf16 wo; psum->sbuf copies on ACT (idle in C);
    bf16 output DMA (host upcasts)
"""

import math
import numpy as np
import ml_dtypes

B, S, D = 2, 2048, 2048
H, KVH = 16, 8
Dh = 64
TP = 4
NCORES = 8
LAYER_IDX = 2
LAMBDA_INIT = 0.8 - 0.6 * math.exp(-0.3 * LAYER_IDX)
EPS = 1e-5
ROPE_THETA = 10000.0

_CACHE = {}


def _build_nc():
    import concourse.bass as bass  # noqa: F401
    import concourse.tile as tile
    from concourse import bacc, mybir

    F32 = mybir.dt.float32
    F32R = mybir.dt.float32r
    BF16 = mybir.dt.bfloat16
    Act = mybir.ActivationFunctionType
    Alu = mybir.AluOpType

    nc = bacc.Bacc("TRN2", target_bir_lowering=False, debug=False)

    xT = nc.dram_tensor("xT", [D, S], BF16, kind="ExternalInput")
    wqT = nc.dram_tensor("wqT", [D, 512], BF16, kind="ExternalInput")
    wkT = nc.dram_tensor("wkT", [D, 256], BF16, kind="ExternalInput")
    wvT = nc.dram_tensor("wvT", [D, 256], BF16, kind="ExternalInput")
    woT = nc.dram_tensor("woT", [512, D], BF16, kind="ExternalInput")
    cosT_d = nc.dram_tensor("cosT", [128, S], F32, kind="ExternalInput")
    ssinT_d = nc.dram_tensor("ssinT", [128, S], F32, kind="ExternalInput")
    maskT_d = nc.dram_tensor("maskT", [128, 128], F32R, kind="ExternalInput")
    identC_d = nc.dram_tensor("identC", [128, 256], F32R, kind="ExternalInput")
    ident_d = nc.dram_tensor("ident", [128, 128], F32, kind="ExternalInput")
    onescol_d = nc.dram_tensor("onescol", [128, 1], BF16, kind="ExternalInput")
    sel4_d = nc.dram_tensor("sel4", [4, 512], BF16, kind="ExternalInput")
    lam_d = nc.dram_tensor("lam", [128, 1], F32, kind="ExternalInput")
    out_d = nc.dram_tensor("out", [S, D], BF16, kind="ExternalOutput")

    KD = D // 128  # 16 contraction tiles
    SHUF = [(i + 16) % 32 for i in range(32)]
    SQ128E = 128.0 * EPS

    with tile.TileContext(nc) as tc:
        with tc.tile_pool(name="const", bufs=1) as constp, \
             tc.tile_pool(name="persist", bufs=1) as persist, \
             tc.tile_pool(name="xtp", bufs=8) as xtp, \
             tc.tile_pool(name="ropet", bufs=3) as rp, \
             tc.tile_pool(name="etp", bufs=4) as etp, \
             tc.tile_pool(name="ebp", bufs=2) as ebp, \
             tc.tile_pool(name="outp", bufs=2) as outp, \
             tc.tile_pool(name="psS", bufs=2, space="PSUM") as psS, \
             tc.tile_pool(name="psOT", bufs=1, space="PSUM") as psOT, \
             tc.tile_pool(name="psSm", bufs=1, space="PSUM") as psSm, \
             tc.tile_pool(name="psBG", bufs=1, space="PSUM") as psBG:

            cosT = constp.tile([128, S], F32, tag="cos")
            ssinT = constp.tile([128, S], F32, tag="ssin")
            maskT = constp.tile([128, 128], F32R, tag="mask")
            identC = constp.tile([128, 256], F32R, tag="idc")
            ident = constp.tile([128, 128], F32, tag="id")
            onescol = constp.tile([128, 1], BF16, tag="onc")
            sel4 = constp.tile([4, 512], BF16, tag="sel4")
            lam = constp.tile([128, 1], F32, tag="lam")

            wq_sb = persist.tile([128, KD * 512], BF16, tag="wq")
            wk_sb = persist.tile([128, KD * 256], BF16, tag="wk")
            wv_sb = persist.tile([128, KD * 256], BF16, tag="wv")
            wo_sb = persist.tile([128, 4 * 2048], BF16, tag="wo")

            qT_sb = [persist.tile([128, S], BF16, tag=f"qT{m}", name=f"qT{m}")
                     for m in range(4)]
            kTd = [persist.tile([128, S], BF16, tag=f"kTd{p}", name=f"kTd{p}")
                   for p in range(4)]
            v_sb = [persist.tile([128, 256], BF16, tag=f"v{ms}", name=f"v{ms}")
                    for ms in range(16)]
            otf = [persist.tile([128, S], BF16, tag=f"otf{p}", name=f"otf{p}")
                   for p in range(4)]
            pre_all = persist.tile([128, 64], F32, tag="pre")
            sf_all = persist.tile([128, 64], F32, tag="sf")

            wqv = wq_sb[:].rearrange("p (kd n) -> p kd n", kd=KD)
            wkv = wk_sb[:].rearrange("p (kd n) -> p kd n", kd=KD)
            wvv = wv_sb[:].rearrange("p (kd n) -> p kd n", kd=KD)
            wov = wo_sb[:].rearrange("p (kc n) -> p kc n", kc=4)

            xt_tiles = {}

            def load_x_one(sh, kp):
                c0 = 512 * sh
                t = xtp.tile([128, 2048], BF16, tag="xt", name=f"xt{sh}_{kp}")
                nc.sync.dma_start(
                    out=t[:].rearrange("p (four n) -> p four n", four=4),
                    in_=xT[kp * 512:kp * 512 + 512, c0:c0 + 512]
                        .rearrange("(four p) n -> p four n", four=4),
                )
                xt_tiles[sh, kp] = t

            def load_x(sh):
                for kp in range(4):
                    load_x_one(sh, kp)

            def xt_rhs(sh, kd):
                return xt_tiles[sh, kd // 4][:, (kd % 4) * 512:(kd % 4) * 512 + 512]

            def load_wq_quarter(qt):
                nc.sync.dma_start(
                    out=wqv[:, qt * 4:(qt + 1) * 4, :],
                    in_=wqT[qt * 512:(qt + 1) * 512, :]
                        .rearrange("(kd p) n -> p kd n", kd=4),
                )

            load_x_one(0, 0)
            load_wq_quarter(0)
            load_x_one(0, 1)
            load_wq_quarter(1)
            nc.sync.dma_start(out=cosT[:], in_=cosT_d[:])
            load_x_one(0, 2)
            load_wq_quarter(2)
            nc.sync.dma_start(out=ssinT[:], in_=ssinT_d[:])
            load_x_one(0, 3)
            load_wq_quarter(3)
            nc.sync.dma_start(
                out=wkv[:], in_=wkT[:].rearrange("(kd p) n -> p kd n", kd=KD))
            nc.sync.dma_start(
                out=wvv[:], in_=wvT[:].rearrange("(kd p) n -> p kd n", kd=KD))
            load_x(1)
            nc.sync.dma_start(out=maskT[:], in_=maskT_d[:])
            nc.sync.dma_start(out=identC[:], in_=identC_d[:])
            nc.sync.dma_start(out=ident[:], in_=ident_d[:])
            nc.sync.dma_start(out=onescol[:], in_=onescol_d[:])
            nc.sync.dma_start(out=sel4[:], in_=sel4_d[:])
            nc.sync.dma_start(out=lam[:], in_=lam_d[:])
            nc.sync.dma_start(
                out=wov[:], in_=woT[:].rearrange("(kc p) n -> p kc n", kc=4))

            # ============ emission helpers ============
            def rope_q(m, qps, csl):
                qsw = rp.tile([128, 512], F32, tag="sw", name="qsw")
                nc.vector.stream_shuffle(qsw[:], qps[:, 0:512], SHUF)
                qc = rp.tile([128, 512], F32, tag="qc", name="qc")
                nc.vector.tensor_mul(qc[:], qps[:, 0:512], cosT[:, csl])
                nc.vector.tensor_mul(qsw[:], qsw[:], ssinT[:, csl])
                nc.vector.tensor_add(qT_sb[m][:, csl], qc[:], qsw[:])

            def rope_k(m, kps, csl):
                ksw = rp.tile([128, 512], F32, tag="sw", name="ksw")
                nc.vector.stream_shuffle(ksw[:], kps[:, 0:512], SHUF)
                kc = rp.tile([128, 512], F32, tag="qc", name="kc")
                nc.vector.tensor_mul(kc[:], kps[:, 0:512], cosT[:, csl])
                nc.vector.tensor_mul(ksw[:], ksw[:], ssinT[:, csl])
                for e in range(2):
                    esl = slice(e * 64, e * 64 + 64)
                    for hf in range(2):
                        nc.vector.tensor_add(
                            kTd[2 * m + e][hf * 64:hf * 64 + 64, csl],
                            kc[esl, :], ksw[esl, :])

            def proj_tiles(sh, pool):
                """yield thunk lists for the 10 projection tiles of shard sh."""
                c0 = 512 * sh
                csl = slice(c0, c0 + 512)
                for kind, m in ([("q", m) for m in range(4)]
                                + [("k", m) for m in range(2)]
                                + [("v", ms) for ms in range(4)]):
                    box = {}

                    def mk(kind, m, box):
                        def alloc():
                            box["ps"] = pool.tile([128, 512], F32, tag="bg",
                                                  name=f"bg{kind}{m}")
                        def mmgrp(g):
                            def f():
                                if g == 0:
                                    alloc()
                                ps = box["ps"]
                                for kd in range(4 * g, 4 * g + 4):
                                    if kind == "q":
                                        nc.tensor.matmul(
                                            ps[:, 0:512],
                                            wqv[:, kd, m * 128:m * 128 + 128],
                                            xt_rhs(sh, kd),
                                            start=(kd == 0), stop=(kd == KD - 1))
                                    elif kind == "k":
                                        nc.tensor.matmul(
                                            ps[:, 0:512],
                                            wkv[:, kd, m * 128:m * 128 + 128],
                                            xt_rhs(sh, kd),
                                            start=(kd == 0), stop=(kd == KD - 1))
                                    else:
                                        nc.tensor.matmul(
                                            ps[:, 0:256],
                                            xt_rhs(sh, kd)[:, m * 128:m * 128 + 128],
                                            wvv[:, kd, :],
                                            start=(kd == 0), stop=(kd == KD - 1))
                                if g == 3:
                                    ps = box["ps"]
                                    if kind == "q":
                                        rope_q(m, ps, csl)
                                    elif kind == "k":
                                        rope_k(m, ps, csl)
                                    else:
                                        nc.vector.tensor_copy(
                                            v_sb[sh * 4 + m][:], ps[:, 0:256])
                            return f
                        return [mmgrp(g) for g in range(4)]
                    yield from mk(kind, m, box)

            def proj_direct(sh):
                for th in proj_tiles(sh, psBG):
                    th()

            bg_queue = []

            def enqueue_proj(sh):
                if (sh, 0) not in xt_tiles:
                    load_x(sh)
                for th in proj_tiles(sh, psBG):
                    bg_queue.append((f"proj{sh}", th))

            def oproj_m(m, pool, copy_eng):
                osb = outp.tile([128, 2048], BF16, tag="ob", name="osb")

                def nblk(n):
                    def f():
                        if pool is psBG:
                            ps = pool.tile([128, 512], F32, tag="bg", name="pc")
                        else:
                            ps = pool.tile([128, 1024], F32, tag="s", name="pc")
                        psv = ps[:, 0:512]
                        for kc in range(4):
                            nc.tensor.matmul(
                                psv,
                                otf[kc][:, m * 128:m * 128 + 128],
                                wov[:, kc, n * 512:n * 512 + 512],
                                start=(kc == 0), stop=(kc == 3),
                            )
                        if copy_eng[n % 2] == "act":
                            nc.scalar.copy(osb[:, n * 512:n * 512 + 512], psv)
                        else:
                            nc.vector.tensor_copy(osb[:, n * 512:n * 512 + 512], psv)
                        if n == 3:
                            nc.sync.dma_start(
                                out=out_d[m * 128:m * 128 + 128, :], in_=osb[:])
                    return f
                return [nblk(n) for n in range(4)]

            def enqueue_oproj(g):
                for m in range(4 * g, 4 * g + 4):
                    for th in oproj_m(m, psBG, ("dve", "dve")):
                        bg_queue.append((f"oproj{g}", th))

            def oproj_direct(g):
                for m in range(4 * g, 4 * g + 4):
                    for th in oproj_m(m, psS, ("act", "dve")):
                        th()

            def pump(n):
                for _ in range(n):
                    if not bg_queue:
                        return
                    bg_queue.pop(0)[1]()

            def flush_tag(tag):
                rest = []
                for t, th in bg_queue:
                    if t == tag:
                        th()
                    else:
                        rest.append((t, th))
                bg_queue[:] = rest

            def flush_all_bg():
                while bg_queue:
                    bg_queue.pop(0)[1]()

            # ============ attention ============
            def emit_ki_loop(qi, p):
                vh = p // 2
                q0 = 512 * qi
                kis = list(range(4 * qi, 4 * qi + 4)) + list(range(4 * qi))
                OT = psOT.tile([128, 1024], F32, tag="ot", name="OT")
                OTv = OT[:].rearrange("p (two n) -> p two n", two=2)
                smalls = psSm.tile([128, 512], F32, tag="sm", name="smalls")
                nc.vector.memset(smalls[:, 0:128], 0.0)
                nki = len(kis)
                for idx, ki in enumerate(kis):
                    j = ki - 4 * qi
                    diag = j >= 0
                    vc = 128 * j if diag and j > 0 else 0
                    ksl = slice(ki * 128, ki * 128 + 128)
                    S12 = psS.tile([128, 1024], F32, tag="s", name="S12")
                    S12v = S12[:].rearrange("p (two n) -> p two n", two=2)
                    for h in range(2):
                        hsl = slice(h * 64, h * 64 + 64)
                        nc.tensor.matmul(
                            S12v[:, h, vc:512],
                            kTd[p][hsl, ksl],
                            qT_sb[p][hsl, q0 + vc:q0 + 512],
                            start=True, stop=not diag,
                        )
                    if diag:
                        nc.tensor.matmul(
                            S12v[:, :, vc:vc + 128], maskT[:], identC[:],
                            start=False, stop=True,
                        )
                    ET = etp.tile([128, 1024], BF16, tag="e", name="ET")
                    ETv = ET[:].rearrange("p (two n) -> p two n", two=2)
                    nc.scalar.activation(ETv[:, :, vc:512], S12v[:, :, vc:512],
                                         Act.Exp)
                    vt = v_sb[ki][:, vh * 128:vh * 128 + 128]
                    st = idx == 0
                    sp = idx == nki - 1
                    for h in range(2):
                        nc.tensor.matmul(OTv[:, h, vc:512], vt,
                                         ETv[:, h, vc:512], start=st, stop=sp)
                    for h in range(2):
                        for c in range(4):
                            if diag and c < j:
                                continue
                            col = h * 64 + c * 16 + idx
                            nc.tensor.matmul(
                                smalls[:, col:col + 1],
                                ETv[:, h, c * 128:c * 128 + 128],
                                onescol[:],
                                start=True, stop=True,
                            )
                    pump(1)
                OTs = ebp.tile([128, 1024], F32, tag="ots", name="OTs")
                nc.vector.tensor_copy(OTs[:, 0:512], OTv[:, 0, :])
                nc.vector.tensor_copy(OTs[:, 512:1024], OTv[:, 1, :])
                Rred = ebp.tile([128, 8], F32, tag="rred", name="Rred")
                nc.vector.tensor_reduce(
                    Rred[:],
                    smalls[:, 0:128].rearrange("p (hc k) -> p hc k", k=16),
                    mybir.AxisListType.X, Alu.add)
                rcp2 = ebp.tile([128, 4], F32, tag="rcp", name="rcp2")
                nc.vector.reciprocal(rcp2[:], Rred[:, 4:8])
                m_q = ebp.tile([128, 4], F32, tag="mq", name="m_q")
                nc.vector.scalar_tensor_tensor(
                    m_q[:], Rred[:, 0:4], lam[:, 0:1], rcp2[:],
                    Alu.mult, Alu.mult)
                t2 = ebp.tile([128, 4], F32, tag="t2", name="t2")
                nc.vector.scalar_tensor_tensor(
                    t2[:], Rred[:, 0:4], SQ128E, Rred[:, 0:4],
                    Alu.mult, Alu.mult)
                return (qi, p, OTs, m_q, t2)

            def emit_late_epilogue(ctx):
                qi, p, OTs, m_q, t2 = ctx
                q0 = 512 * qi
                m_b = psS.tile([128, 1024], F32, tag="s", name="m_b")
                nc.tensor.transpose(m_b[0:4, 640:768], m_q[:], ident[:])
                mrow = ebp.tile([4, 128], BF16, tag="mrow", name="mrow")
                nc.vector.tensor_copy(mrow[:], m_b[0:4, 640:768])
                for c in range(4):
                    nc.tensor.matmul(m_b[:, c * 128:c * 128 + 128],
                                     sel4[:, c * 128:c * 128 + 128], mrow[:],
                                     start=True, stop=True)
                tt = ebp.tile([128, 512], F32, tag="tt", name="tt")
                nc.vector.tensor_mul(tt[:], OTs[:, 512:1024], m_b[:, 0:512])
                nc.vector.tensor_sub(otf[p][:, q0:q0 + 512], OTs[:, 0:512], tt[:])
                sq = ebp.tile([128, 512], BF16, tag="sq", name="sq")
                nc.vector.tensor_mul(sq[:], otf[p][:, q0:q0 + 512],
                                     otf[p][:, q0:q0 + 512])
                for c in range(4):
                    nc.tensor.matmul(m_b[:, 512 + c:513 + c],
                                     sq[:, c * 128:c * 128 + 128],
                                     onescol[:], start=True, stop=True)
                off = (qi * 4 + p) * 4
                nc.vector.tensor_add(pre_all[:, off:off + 4],
                                     m_b[:, 512:516], t2[:])

            pending = [None]

            def attn_group(qi):
                for p in range(4):
                    ctx = emit_ki_loop(qi, p)
                    if pending[0] is not None:
                        emit_late_epilogue(pending[0])
                    pending[0] = ctx
                    pump(2)

            def flush_pending():
                if pending[0] is not None:
                    emit_late_epilogue(pending[0])
                    pending[0] = None

            def bfive_group(qi):
                goff = qi * 16
                lnp = ebp.tile([128, 16], F32, tag="lnp", name="lnp")
                nc.scalar.activation(lnp[:], pre_all[:, goff:goff + 16],
                                     Act.Ln, scale=1.0 / 128.0)
                nc.scalar.activation(sf_all[:, goff:goff + 16], lnp[:],
                                     Act.Exp, scale=-0.5)
                q0 = 512 * qi
                for p in range(4):
                    off = goff + p * 4
                    sf_b = psS.tile([128, 1024], F32, tag="s", name="sf_b")
                    nc.tensor.transpose(sf_b[0:4, 640:768],
                                        sf_all[:, off:off + 4], ident[:])
                    sfrow = ebp.tile([4, 128], BF16, tag="mrow", name="sfrow")
                    nc.vector.tensor_copy(sfrow[:], sf_b[0:4, 640:768])
                    for c in range(4):
                        nc.tensor.matmul(sf_b[:, c * 128:c * 128 + 128],
                                         sel4[:, c * 128:c * 128 + 128], sfrow[:],
                                         start=True, stop=True)
                    nc.vector.tensor_mul(otf[p][:, q0:q0 + 512],
                                         otf[p][:, q0:q0 + 512], sf_b[:, 0:512])

            # ============ interleaved schedule ============
            proj_direct(0)
            proj_direct(1)
            enqueue_proj(2)
            attn_group(0)
            enqueue_proj(3)
            attn_group(1)
            flush_tag("proj2")
            flush_pending()
            bfive_group(0)
            enqueue_oproj(0)
            attn_group(2)
            flush_tag("proj3")
            flush_pending()
            bfive_group(1)
            enqueue_oproj(1)
            attn_group(3)
            flush_all_bg()
            flush_pending()
            bfive_group(2)
            oproj_direct(2)
            bfive_group(3)
            oproj_direct(3)

    nc.compile()
    return nc


def _perm64():
    return np.array(list(range(0, 16)) + list(range(32, 48)) +
                    list(range(16, 32)) + list(range(48, 64)))


def _host_tables():
    p64 = _perm64()
    inv = ROPE_THETA ** (-np.arange(Dh, dtype=np.float64) / Dh)
    pos = np.arange(S, dtype=np.float64)
    fr = pos[:, None] * inv[None, :]              # [S, 64]
    cos = np.cos(fr).astype(np.float32)           # [S, 64]
    sin = np.sin(fr).astype(np.float32)
    d = p64[np.arange(128) % 64]
    cosT = np.ascontiguousarray(cos[:, d].T)      # [128, S]
    sgn = np.where(d < 32, -1.0, 1.0).astype(np.float32)
    ssinT = np.ascontiguousarray(sin[:, d].T * sgn[:, None])
    maskT = np.triu(np.full((128, 128), -50.0, np.float32), 1)
    identC = np.ascontiguousarray(
        np.concatenate([np.eye(128, dtype=np.float32)] * 2, axis=1))
    ident = np.eye(128, dtype=np.float32)
    onescol = np.ones((128, 1), np.float32).astype(ml_dtypes.bfloat16)
    sel4 = np.zeros((4, 512), np.float32)
    for c in range(4):
        sel4[c, c * 128:(c + 1) * 128] = 1.0
    sel4 = sel4.astype(ml_dtypes.bfloat16)
    return cosT, ssinT, maskT, identC, ident, onescol, sel4


def kernel(hidden_states, Wq, Wk, Wv, Wo,
           lambda_q1, lambda_k1, lambda_q2, lambda_k2, subln_weight):
    from concourse.bass_utils import run_bass_kernel_spmd

    if "nc" not in _CACHE:
        _CACHE["nc"] = _build_nc()
        _CACHE["tables"] = _host_tables()
    nc = _CACHE["nc"]
    cosT, ssinT, maskT, identC, ident, onescol, sel4 = _CACHE["tables"]

    f32 = np.float32
    bf16 = ml_dtypes.bfloat16
    hs = np.asarray(hidden_states, f32)
    Wq = np.asarray(Wq, f32)
    Wk = np.asarray(Wk, f32)
    Wv = np.asarray(Wv, f32)
    Wo = np.asarray(Wo, f32)
    subln = np.asarray(subln_weight, f32)

    lam1 = np.exp(np.sum(np.asarray(lambda_q1, f32) * np.asarray(lambda_k1, f32),
                         dtype=f32))
    lam2 = np.exp(np.sum(np.asarray(lambda_q2, f32) * np.asarray(lambda_k2, f32),
                         dtype=f32))
    lam_full = f32(lam1 - lam2 + LAMBDA_INIT)
    lam_arr = np.full((128, 1), lam_full, f32)

    scale = f32(Dh ** -0.5)
    wprime = (np.tile(subln, H) * f32(1.0 - LAMBDA_INIT)).astype(f32)  # [2048]
    WoS = Wo * wprime[None, :]

    p64 = _perm64()
    qperm = (np.repeat(np.arange(8) * 64, 64) + np.tile(p64, 8))
    kperm = (np.repeat(np.arange(4) * 64, 64) + np.tile(p64, 4))

    in_maps = []
    for c in range(NCORES):
        b, r = c // TP, c % TP
        wq_h = np.ascontiguousarray(
            (Wq[512 * r:512 * r + 512, :] * scale).T[:, qperm]).astype(bf16)
        wk_h = np.ascontiguousarray(
            Wk[256 * r:256 * r + 256, :].T[:, kperm]).astype(bf16)
        wv_h = np.ascontiguousarray(Wv[256 * r:256 * r + 256, :].T).astype(bf16)
        wo_h = np.ascontiguousarray(WoS[:, 512 * r:512 * r + 512].T).astype(bf16)
        in_maps.append({
            "xT": np.ascontiguousarray(hs[b].T).astype(bf16),
            "wqT": wq_h, "wkT": wk_h, "wvT": wv_h, "woT": wo_h,
            "cosT": cosT, "ssinT": ssinT, "maskT": maskT, "identC": identC,
            "ident": ident, "onescol": onescol, "sel4": sel4,
            "lam": lam_arr,
        })

    res = run_bass_kernel_spmd(nc, in_maps, core_ids=list(range(NCORES)))
    out = np.zeros((B, S, D), f32)
    for c in range(NCORES):
        out[c // TP] += np.asarray(res.results[c]["out"]).astype(f32)
    return out
